# revision 10
# baseline (speedup 1.0000x reference)
"""Trainium2 Bass kernel for ExpanderLinear: out = x @ (W * mask).T

Shapes (hardcoded): x [8192, 4096] f32, weight [4096, 4096] f32,
mask [4096, 4096] f32 -> out [8192, 4096] f32.

Strategy: tensor-parallel over output features across 8 cores. The host
pre-transposes the operands (input marshalling, like GEMM pre-packing):
  xT [4096, 8192], wT/maskT column slices [4096, 512] per core.
Each core computes outT_c = (W_c*mask_c) @ x.T as [512, 8192]; the host
transposes/concatenates.

Per-core device kernel (float32r matmuls: 1 cycle/row at N=512,
~1.5e-4 scale-relative error):
  - wmT = round_f32r(wT_c * maskT_c) on DVE -> [128, 32, 512] SBUF.
  - per 512-col chunk of xT: DMA -> SBUF, DVE round to f32r sub-tiles,
    then 4 x 32 accumulating matmuls into psum [128 o, 512 b],
    lhsT = wmT chunk (stationary), rhs = xT chunk (moving).
No PE transposes: the tensor engine runs matmuls only.
"""

import ml_dtypes
import numpy as np

import concourse.bass as bass
import concourse.mybir as mybir
import concourse.tile as tile
from concourse import bacc
from concourse.bass_utils import run_bass_kernel_spmd

P = 128
D_IN = 4096
D_OUT = 4096
BATCH = 8192
N_CORES = 8
O_PER_CORE = D_OUT // N_CORES  # 512
KC = D_IN // P  # 32 contraction chunks
B_CHUNK = 512
N_BCHUNK = BATCH // B_CHUNK  # 16
OT = O_PER_CORE // P  # 4 output partition tiles
KG = 4  # ic groups per chunk
KCG = KC // KG  # 8 ics per group

F32 = mybir.dt.float32
F32R = mybir.dt.float32r
BF16 = mybir.dt.bfloat16


def build_nc():
    nc = bacc.Bacc("TRN2", target_bir_lowering=False, debug=False, num_devices=N_CORES)

    xT_d = nc.dram_tensor("xT", [D_IN, BATCH], F32, kind="ExternalInput")
    wT_d = nc.dram_tensor("wT", [D_IN, O_PER_CORE], F32, kind="ExternalInput")
    mT_d = nc.dram_tensor("maskT", [D_IN, O_PER_CORE], BF16, kind="ExternalInput")
    outT_d = nc.dram_tensor("outT", [O_PER_CORE, BATCH], F32, kind="ExternalOutput")

    with tile.TileContext(nc) as tc:
        with (
            tc.tile_pool(name="persist", bufs=1) as persist,
            tc.tile_pool(name="stage", bufs=3) as stage,
            tc.tile_pool(name="xr", bufs=KG + 1) as xrpool,
            tc.tile_pool(name="outp", bufs=2) as outp,
            tc.tile_pool(name="mpsum", bufs=8, space="PSUM") as mpsum,
        ):
            # --- WmT prep: 8 eighth tiles [128, KC//8, 512] f32r, finely
            # interleaved with bc0's x loads so the first matmul starts
            # as soon as ~7 MB have landed ---
            NWE = 8
            WPE = KC // NWE  # 4 ics per eighth
            wmT_e = []

            def emit_wm_eighth(e):
                r_sl = slice(e * WPE * P, (e + 1) * WPE * P)
                w_t = stage.tile([P, WPE, O_PER_CORE], F32, tag="s", name=f"w{e}")
                m_t = stage.tile([P, WPE, O_PER_CORE], BF16, tag="s", name=f"m{e}")
                nc.sync.dma_start(
                    w_t, wT_d[r_sl, :].rearrange("(kc p) o -> p kc o", p=P)
                )
                nc.sync.dma_start(
                    m_t, mT_d[r_sl, :].rearrange("(kc p) o -> p kc o", p=P)
                )
                wm = persist.tile([P, WPE, O_PER_CORE], F32R, name=f"wmT{e}")
                # mask-multiply with f32r rounding fused into the output dtype
                nc.vector.tensor_mul(wm, w_t, m_t)
                wmT_e.append(wm)

            def emit_x_sub(bc, g):
                xs = stage.tile([P, KCG, B_CHUNK], F32, tag="s", name="xs")
                rows = slice(g * (D_IN // KG), (g + 1) * (D_IN // KG))
                cols = slice(bc * B_CHUNK, (bc + 1) * B_CHUNK)
                nc.sync.dma_start(
                    xs, xT_d[rows, cols].rearrange("(kc p) b -> p kc b", p=P)
                )
                xr = xrpool.tile([P, KCG, B_CHUNK], F32R, tag="xr", name="xr")
                nc.vector.tensor_copy(xr, xs)  # f32r rounding
                return xr

            pending = []
            for e in range(NWE):
                emit_wm_eighth(e)
                if e % 2 == 0:
                    pending.append(emit_x_sub(0, e // 2))

            def lhsT(ic, oc):
                return wmT_e[ic // WPE][:, ic % WPE, oc * P : (oc + 1) * P]

            # --- main loop over batch chunks ---
            for bc in range(N_BCHUNK):
                xr_subs = pending
                psums = [
                    mpsum.tile([P, B_CHUNK], F32, name=f"ps{oc}", tag="ps")
                    for oc in range(OT)
                ]
                last = bc == N_BCHUNK - 1
                if last:
                    # oc-major so each psum finishes early and its drain +
                    # output DMA overlap the remaining matmuls (shorter tail)
                    for oc in range(OT):
                        for g in range(KG):
                            for k in range(KCG):
                                ic = g * KCG + k
                                nc.tensor.matmul(
                                    psums[oc],
                                    lhsT(ic, oc),
                                    xr_subs[g][:, k, :],
                                    start=(ic == 0),
                                    stop=(ic == KC - 1),
                                )
                        ob = outp.tile([P, B_CHUNK], F32)
                        nc.vector.tensor_copy(ob, psums[oc])
                        nc.sync.dma_start(
                            outT_d[
                                oc * P : (oc + 1) * P,
                                bc * B_CHUNK : (bc + 1) * B_CHUNK,
                            ],
                            ob,
                        )
                    continue
                for g in range(KG):
                    for k in range(KCG):
                        ic = g * KCG + k
                        for oc in range(OT):
                            nc.tensor.matmul(
                                psums[oc],
                                lhsT(ic, oc),
                                xr_subs[g][:, k, :],
                                start=(ic == 0),
                                stop=(ic == KC - 1),
                            )
                if bc + 1 < N_BCHUNK:
                    pending = [emit_x_sub(bc + 1, g) for g in range(KG)]
                for oc in range(OT):
                    ob = outp.tile([P, B_CHUNK], F32)
                    nc.vector.tensor_copy(ob, psums[oc])
                    nc.sync.dma_start(
                        outT_d[
                            oc * P : (oc + 1) * P, bc * B_CHUNK : (bc + 1) * B_CHUNK
                        ],
                        ob,
                    )

    nc.compile()
    return nc


_NC_CACHE = None


def _shard_inputs(x, weight, mask):
    """Host-side marshalling: transpose operands and slice per core."""
    x = np.asarray(x, dtype=np.float32)
    weight = np.asarray(weight, dtype=np.float32)
    mask = np.asarray(mask, dtype=np.float32)
    xT = np.ascontiguousarray(x.T)
    wT = weight.T
    mT = mask.T
    in_maps = []
    for c in range(N_CORES):
        sl = slice(c * O_PER_CORE, (c + 1) * O_PER_CORE)
        in_maps.append(
            {
                "xT": xT,
                "wT": np.ascontiguousarray(wT[:, sl]),
                "maskT": np.ascontiguousarray(mT[:, sl]).astype(ml_dtypes.bfloat16),
            }
        )
    return in_maps


def kernel(x, weight, mask):
    global _NC_CACHE
    if _NC_CACHE is None:
        _NC_CACHE = build_nc()
    nc = _NC_CACHE

    in_maps = _shard_inputs(x, weight, mask)
    res = run_bass_kernel_spmd(nc, in_maps, core_ids=list(range(N_CORES)))

    out = np.empty((BATCH, D_OUT), dtype=np.float32)
    for c in range(N_CORES):
        sl = slice(c * O_PER_CORE, (c + 1) * O_PER_CORE)
        out[:, sl] = res.results[c]["outT"].T
    return out


# revision 11
# speedup vs baseline: 1.2092x; 1.2092x over previous
"""Trainium2 Bass kernel for ExpanderLinear: out = x @ (W * mask).T

Shapes (hardcoded): x [8192, 4096] f32, weight [4096, 4096] f32,
mask [4096, 4096] f32 -> out [8192, 4096] f32.

Strategy: tensor-parallel over output features across 8 cores. The host
pre-transposes the operands (input marshalling, like GEMM pre-packing):
  xT [4096, 8192], wT/maskT column slices [4096, 512] per core.
Each core computes outT_c = (W_c*mask_c) @ x.T as [512, 8192]; the host
transposes/concatenates.

Per-core device kernel (float32r matmuls: 1 cycle/row at N=512,
~1.5e-4 scale-relative error):
  - wmT = round_f32r(wT_c * maskT_c) on DVE -> [128, 32, 512] SBUF.
  - per 512-col chunk of xT: DMA -> SBUF, DVE round to f32r sub-tiles,
    then 4 x 32 accumulating matmuls into psum [128 o, 512 b],
    lhsT = wmT chunk (stationary), rhs = xT chunk (moving).
No PE transposes: the tensor engine runs matmuls only.
"""

import ml_dtypes
import numpy as np

import concourse.bass as bass
import concourse.mybir as mybir
import concourse.tile as tile
from concourse import bacc
from concourse.bass_utils import run_bass_kernel_spmd

P = 128
D_IN = 4096
D_OUT = 4096
BATCH = 8192
N_CORES = 8
O_PER_CORE = D_OUT // N_CORES  # 512
KC = D_IN // P  # 32 contraction chunks
B_CHUNK = 512
N_BCHUNK = BATCH // B_CHUNK  # 16
OT = O_PER_CORE // P  # 4 output partition tiles
KG = 8  # ic groups per chunk
KCG = KC // KG  # 4 ics per group

F32 = mybir.dt.float32
F32R = mybir.dt.float32r
BF16 = mybir.dt.bfloat16


def build_nc():
    nc = bacc.Bacc("TRN2", target_bir_lowering=False, debug=False, num_devices=N_CORES)

    xT_d = nc.dram_tensor("xT", [D_IN, BATCH], F32, kind="ExternalInput")
    wT_d = nc.dram_tensor("wT", [D_IN, O_PER_CORE], F32, kind="ExternalInput")
    mT_d = nc.dram_tensor("maskT", [D_IN, O_PER_CORE], BF16, kind="ExternalInput")
    outT_d = nc.dram_tensor("outT", [O_PER_CORE, BATCH], F32, kind="ExternalOutput")

    with tile.TileContext(nc) as tc:
        with (
            tc.tile_pool(name="persist", bufs=1) as persist,
            tc.tile_pool(name="stage", bufs=4) as stage,
            tc.tile_pool(name="xr", bufs=12) as xrpool,
            tc.tile_pool(name="outp", bufs=2) as outp,
            tc.tile_pool(name="mpsum", bufs=8, space="PSUM") as mpsum,
        ):
            # --- WmT prep: 8 eighth tiles [128, KC//8, 512] f32r, finely
            # interleaved with bc0's x loads so the first matmul starts
            # as soon as ~7 MB have landed ---
            NWE = 8
            WPE = KC // NWE  # 4 ics per eighth
            wmT_e = []

            def emit_wm_eighth(e):
                r_sl = slice(e * WPE * P, (e + 1) * WPE * P)
                w_t = stage.tile([P, WPE, O_PER_CORE], F32, tag="s", name=f"w{e}")
                m_t = stage.tile([P, WPE, O_PER_CORE], BF16, tag="s", name=f"m{e}")
                nc.sync.dma_start(
                    w_t, wT_d[r_sl, :].rearrange("(kc p) o -> p kc o", p=P)
                )
                nc.sync.dma_start(
                    m_t, mT_d[r_sl, :].rearrange("(kc p) o -> p kc o", p=P)
                )
                wm = persist.tile([P, WPE, O_PER_CORE], F32R, name=f"wmT{e}")
                # mask-multiply with f32r rounding fused into the output dtype
                nc.vector.tensor_mul(wm, w_t, m_t)
                wmT_e.append(wm)

            def emit_x_sub(bc, g):
                xs = stage.tile([P, KCG, B_CHUNK], F32, tag="s", name="xs")
                rows = slice(g * (D_IN // KG), (g + 1) * (D_IN // KG))
                cols = slice(bc * B_CHUNK, (bc + 1) * B_CHUNK)
                nc.sync.dma_start(
                    xs, xT_d[rows, cols].rearrange("(kc p) b -> p kc b", p=P)
                )
                xr = xrpool.tile([P, KCG, B_CHUNK], F32R, tag="xr", name="xr")
                nc.vector.tensor_copy(xr, xs)  # f32r rounding
                return xr

            pending = []
            for e in range(NWE):
                emit_wm_eighth(e)
                pending.append(emit_x_sub(0, e))

            def lhsT(ic, oc):
                return wmT_e[ic // WPE][:, ic % WPE, oc * P : (oc + 1) * P]

            # --- main loop over batch chunks ---
            for bc in range(N_BCHUNK):
                xr_subs = pending
                psums = [
                    mpsum.tile([P, B_CHUNK], F32, name=f"ps{oc}", tag="ps")
                    for oc in range(OT)
                ]
                last = bc == N_BCHUNK - 1
                if last:
                    # oc-major so each psum finishes early and its drain +
                    # output DMA overlap the remaining matmuls (shorter tail)
                    for oc in range(OT):
                        for g in range(KG):
                            for k in range(KCG):
                                ic = g * KCG + k
                                nc.tensor.matmul(
                                    psums[oc],
                                    lhsT(ic, oc),
                                    xr_subs[g][:, k, :],
                                    start=(ic == 0),
                                    stop=(ic == KC - 1),
                                )
                        ob = outp.tile([P, B_CHUNK], F32)
                        nc.vector.tensor_copy(ob, psums[oc])
                        nc.sync.dma_start(
                            outT_d[
                                oc * P : (oc + 1) * P,
                                bc * B_CHUNK : (bc + 1) * B_CHUNK,
                            ],
                            ob,
                        )
                    continue
                for g in range(KG):
                    for k in range(KCG):
                        ic = g * KCG + k
                        for oc in range(OT):
                            nc.tensor.matmul(
                                psums[oc],
                                lhsT(ic, oc),
                                xr_subs[g][:, k, :],
                                start=(ic == 0),
                                stop=(ic == KC - 1),
                            )
                if bc + 1 < N_BCHUNK:
                    pending = [emit_x_sub(bc + 1, g) for g in range(KG)]
                for oc in range(OT):
                    ob = outp.tile([P, B_CHUNK], F32)
                    nc.vector.tensor_copy(ob, psums[oc])
                    nc.sync.dma_start(
                        outT_d[
                            oc * P : (oc + 1) * P, bc * B_CHUNK : (bc + 1) * B_CHUNK
                        ],
                        ob,
                    )

    nc.compile()
    return nc


_NC_CACHE = None


def _shard_inputs(x, weight, mask):
    """Host-side marshalling: transpose operands and slice per core."""
    x = np.asarray(x, dtype=np.float32)
    weight = np.asarray(weight, dtype=np.float32)
    mask = np.asarray(mask, dtype=np.float32)
    xT = np.ascontiguousarray(x.T)
    wT = weight.T
    mT = mask.T
    in_maps = []
    for c in range(N_CORES):
        sl = slice(c * O_PER_CORE, (c + 1) * O_PER_CORE)
        in_maps.append(
            {
                "xT": xT,
                "wT": np.ascontiguousarray(wT[:, sl]),
                "maskT": np.ascontiguousarray(mT[:, sl]).astype(ml_dtypes.bfloat16),
            }
        )
    return in_maps


def kernel(x, weight, mask):
    global _NC_CACHE
    if _NC_CACHE is None:
        _NC_CACHE = build_nc()
    nc = _NC_CACHE

    in_maps = _shard_inputs(x, weight, mask)
    res = run_bass_kernel_spmd(nc, in_maps, core_ids=list(range(N_CORES)))

    out = np.empty((BATCH, D_OUT), dtype=np.float32)
    for c in range(N_CORES):
        sl = slice(c * O_PER_CORE, (c + 1) * O_PER_CORE)
        out[:, sl] = res.results[c]["outT"].T
    return out
